# revision 22
# baseline (speedup 1.0000x reference)
"""DenseKAN forward as a single fused fp16 matmul on TRN2.

Math: the reference uses a uniform knot grid (spacing h=0.4 on
[-2.2, 2.2]), so the Cox-de Boor bases are shifted copies of the
cardinal cubic B-spline; each basis B_j expands over truncated-power
features f_n = relu(u_n)^3 with

    u_n = (n-1.5)/2.5 - x   (n < 4,  right-side powers)
    u_n = x + (5.5-n)/2.5   (n >= 4, left-side powers)

plus a silu(x) block; all basis coefficients, the per-dim scale factor
and the bias (partition of unity, sum_j B_j == 1) fold into the weights
on the host, so the layer is out = [silu(x) | relu(u)^3 blocks] @ W2.

Pipeline (the original bottleneck was 8 serialized GpSimd tensor_scalar
ops at ~3.8us each, which also degraded concurrent DVE ops ~5x): the
host ships the 8 shifted blocks U = [u_0..u_7] as ONE [128, 2048] fp16
image per core, so on-chip feature work is three WIDE ops per 4-block
chunk — DVE relu (tensor_scalar max), square (ACT for one chunk, DVE
for the other — balances the engines), DVE multiply — no GpSimd.
silu(x) is recovered from u_0 via ACT Silu(-u_0 - 0.6). Everything
(U, features, weights) is fp16: rel err ~2.7e-3 (vs 2e-2 budget),
weight DMA halves vs fp32, and the PE runs at bf16 rate with fast
weight load. DMAs are split across both HWDGE rings (SP + ACT) so
issue costs don't serialize, activation tables are force-loaded early
via dummy ops, and the PE is kept busy with warm-up matmuls so the
HAM clock gate releases before the real matmuls. Batch is sharded
across the 8 cores (128 rows each); weights are replicated.
"""

import numpy as np

import concourse.bass as bass
import concourse.mybir as mybir
import concourse.tile as tile
from concourse import bacc
from concourse.bass_utils import run_bass_kernel_spmd
from concourse.vector_clock import ScopedClock


def _lean_drain_and_barrier(self, tick_clock, wait_clock):
    """TileContext exit without the semaphore range-clear + second
    all-engine barrier: the NEFF epilogue zeroes every semaphore before
    the next execution anyway, so for a single top-level context that
    ends the kernel they only add ~0.4us. The drain + first barrier stay
    (they order every engine behind the last tracked op)."""
    drain_inst = self.nc.sync.drain()
    wait_clock.add_sem_waits(
        drain_inst.ins, ScopedClock({None: tick_clock.global_clock}))
    self.nc.all_engine_barrier()
    popped = self.nc._tile_sem_poison_stack.pop()
    assert popped is self._sem_poison
    sems = list(self.sems.allocated().values())
    sem_nums = [s.num if hasattr(s, "num") else s for s in sems]
    self.nc._state.prepend_free_semaphores(sem_nums)
    for poison_set in self.nc._tile_sem_poison_stack:
        poison_set.update(sem_nums)

BATCH = 1024
IN = 256
UNITS = 256
GK = 8  # number of spline bases per input dim
NF = GK + 1  # + silu feature block
K = IN * NF  # 2304 contraction rows
N_CORES = 8
BS = BATCH // N_CORES  # 128 batch rows per core
KT = K // 128  # 18 k-tiles
N_WARM = 8  # PE warm-up matmuls (HAM clock-gate burn-in)

FP32 = mybir.dt.float32
F16 = mybir.dt.float16

AluOp = mybir.AluOpType
Act = mybir.ActivationFunctionType

_cache = {}


def _build():
    nc = bacc.Bacc("TRN2", target_bir_lowering=False, debug=False,
                   enable_asserts=False, num_devices=N_CORES)
    # host ships the 8 shifted blocks [u_0 | ... | u_7], each [128, 256]
    xu_d = nc.dram_tensor("xu", [128, GK * 256], F16,
                          kind="ExternalInput").ap()
    # host pre-swizzled: w2[p, k, o] = W2_flat[128*k + p, o], fp16
    w_d = nc.dram_tensor("w2", [128, KT, UNITS], F16,
                         kind="ExternalInput").ap()
    o_d = nc.dram_tensor("out", [BS, UNITS], FP32, kind="ExternalOutput").ap()

    # raw (non-Tile) SBUF staging for the output so the final stores can
    # be issued AFTER the TileContext closes: the exit drain then waits
    # only for the PSUM->SBUF copies, and the stores' ~2.4us HBM
    # completion receipt overlaps the NEFF's fixed semaphore-sweep
    # epilogue (~6us) instead of preceding it.
    osb = nc.alloc_sbuf_tensor("osb", [BS, UNITS], FP32).ap()
    # codegen requires sync info on HWDGE DMAs; nothing ever waits on
    # this sem (the NEFF epilogue re-zeroes all semaphores each run)
    s_out = nc.alloc_semaphore("out_fire")

    tc_outer = tile.TileContext(nc)
    tc_outer._drain_and_barrier = _lean_drain_and_barrier.__get__(tc_outer)
    with tc_outer as tc:
        with (
            tc.tile_pool(name="const", bufs=1) as cpool,
            tc.tile_pool(name="blk", bufs=2) as bpool,
            tc.tile_pool(name="psum", bufs=1, space="PSUM") as ppool,
        ):
            # DMAs on both HWDGE rings so issue costs don't serialize.
            # ACT also runs the activation-table loads early, so it only
            # gets the late weight chunks; the final chunk is small so
            # few matmuls trail its completion receipt.
            xu = cpool.tile([128, GK * 256], F16)
            w2 = cpool.tile([128, KT, UNITS], F16)
            nc.sync.dma_start(xu[:], xu_d[:])
            nc.scalar.dma_start(w2[:, 0:2, :], w_d[:, 0:2, :])
            nc.sync.dma_start(w2[:, 2:10, :], w_d[:, 2:10, :])
            nc.scalar.dma_start(w2[:, 10:16, :], w_d[:, 10:16, :])
            nc.sync.dma_start(w2[:, 16:18, :], w_d[:, 16:18, :])

            sbias = cpool.tile([128, 1], FP32)
            nc.vector.memset(sbias[:], -0.6)

            # force the SILU and SQUARE activation-table loads off the
            # critical path: dummy 1-col activations while DMAs stream
            dummy = cpool.tile([128, 1], F16)
            nc.scalar.activation(dummy[:], sbias[:], Act.Silu,
                                 bias=sbias[:], scale=-1.0)
            nc.scalar.square(dummy[:], sbias[:])

            # PE warm-up: HAM keeps the PE at 1.2 GHz until ~3.4us of
            # sustained activity; burn that in while the inputs stream
            wtile = cpool.tile([128, 512], F16)
            nc.vector.tensor_copy(
                wtile[:], nc.const_aps.tensor(1.0, (128, 512), FP32))
            wpsum = ppool.tile([128, 512], FP32)
            for _ in range(N_WARM):
                nc.tensor.matmul(wpsum[:], wtile[:, 0:128], wtile[:],
                                 start=True, stop=True)

            T = cpool.tile([128, NF * 256], F16)
            opsum = ppool.tile([BS, UNITS], FP32)

            # silu(x) = Silu(-u_0 - 0.6); k-tiles 0,1
            nc.scalar.activation(T[:, 0:256], xu[:, 0:256], Act.Silu,
                                 bias=sbias[:], scale=-1.0)
            nc.tensor.matmul(opsum[:], T[:, 0:128], w2[:, 0, :],
                             start=True, stop=False)
            nc.tensor.matmul(opsum[:], T[:, 128:256], w2[:, 1, :],
                             start=False, stop=False)
            # keep the PE's HAM activity window busy until the r-block
            # matmuls arrive, so they run at the unthrottled clock
            for _ in range(4):
                nc.tensor.matmul(wpsum[:], wtile[:, 0:128], wtile[:],
                                 start=True, stop=True)

            # r blocks in two 4-block chunks: relu (DVE), square (ACT
            # for chunk 0, DVE for chunk 1 — balances the engines),
            # multiply (DVE); ops ordered by data readiness so the DVE
            # runs them in arrival order
            uA, uB = xu[:, 0:1024], xu[:, 1024:2048]
            tA = bpool.tile([128, 1024], F16, tag="tA")
            nc.vector.tensor_scalar_max(tA[:], uA, 0.0)
            sA = bpool.tile([128, 1024], F16, tag="sA")
            nc.scalar.square(sA[:], uA)
            tB = bpool.tile([128, 1024], F16, tag="tB")
            nc.vector.tensor_scalar_max(tB[:], uB, 0.0)
            sB = bpool.tile([128, 1024], F16, tag="sB")
            nc.vector.tensor_mul(sB[:], uB, uB)
            nc.vector.tensor_mul(T[:, 256:1280], tA[:], sA[:])
            nc.vector.tensor_mul(T[:, 1280:2304], tB[:], sB[:])
            for kt in range(2, KT):
                nc.tensor.matmul(opsum[:],
                                 T[:, kt * 128:(kt + 1) * 128],
                                 w2[:, kt, :],
                                 start=False, stop=(kt == KT - 1))

            nc.vector.tensor_copy(osb[:], opsum[:])

    # fire-and-forget output store (sem attached but never awaited): the
    # Tile exit barrier above ordered it after the copy, and the walrus
    # epilogue outlasts its completion by several us. One DMA on Sync
    # only, so Scalar — the slowest semaphore-sweep engine — reaches the
    # epilogue barrier as early as possible.
    nc.sync.dma_start(o_d[:], osb[:]).then_inc(s_out, 16)

    nc.compile()
    return nc


def _fold_weights(spline_kernel, scale_factor, bias):
    """-> (128, KT, UNITS) fp16 swizzled folded weights."""
    sk = spline_kernel.astype(np.float64)
    sf = scale_factor.astype(np.float64)
    b = bias.astype(np.float64)
    # W[i,j,o] = sk*sf + bias/IN  (bias folded via sum_j B_j == 1)
    W = sk * sf[:, None, :] + b[None, None, :] / IN
    comb = 2.5 ** 3 * np.array([1.0, -4.0, 6.0, -4.0, 1.0]) / 6.0
    # A[j, n] = coefficient of feature-block n in basis j
    A = np.zeros((GK, GK))
    for j in range(4):  # right-side: B_j = sum_m comb[m] * f_{j-m}
        for m in range(j + 1):
            A[j, j - m] = comb[m]
    for j in range(4, GK):  # left-side: B_j = sum_m comb[m] * f_{j+m}
        for m in range(GK - j):
            A[j, j + m] = comb[m]
    W2 = np.einsum("jn,ijo->nio", A, W)  # (GK, IN, UNITS)
    Wfull = np.concatenate([sf[None, :, :], W2], axis=0)  # silu block first
    flat = Wfull.reshape(K, UNITS)
    sw = flat.reshape(KT, 128, UNITS).transpose(1, 0, 2)  # -> [p, k, o]
    return np.ascontiguousarray(sw.astype(np.float16))


def _prep_x(x):
    """(BATCH, IN) -> per-core (128, GK*256) fp16 images [u_0 | .. | u_7]."""
    x = np.asarray(x, dtype=np.float32)
    outs = []
    for c in range(N_CORES):
        xs = x[c * BS:(c + 1) * BS]  # (BS, IN)
        xtc = np.ascontiguousarray(xs.T)  # (IN, BS)
        img = np.concatenate([xtc[:128], xtc[128:]], axis=1)  # (128, 256)
        blocks = []
        for n in range(GK):
            if n < 4:
                blocks.append((n - 1.5) / 2.5 - img)
            else:
                blocks.append(img + (5.5 - n) / 2.5)
        outs.append(np.ascontiguousarray(
            np.concatenate(blocks, axis=1).astype(np.float16)))
    return outs


def make_in_maps(inputs):
    w2 = _fold_weights(inputs["spline_kernel"], inputs["scale_factor"],
                       inputs["bias"])
    xus = _prep_x(inputs["x"])
    return [{"xu": xus[c], "w2": w2} for c in range(N_CORES)]


def kernel(x, spline_kernel, scale_factor, bias):
    if "nc" not in _cache:
        _cache["nc"] = _build()
    nc = _cache["nc"]

    in_maps = make_in_maps({"x": x, "spline_kernel": spline_kernel,
                            "scale_factor": scale_factor, "bias": bias})
    res = run_bass_kernel_spmd(nc, in_maps, list(range(N_CORES)))
    out = np.concatenate([res.results[c]["out"] for c in range(N_CORES)],
                         axis=0)
    return out.astype(np.float32)
